# revision 30
# baseline (speedup 1.0000x reference)
"""Expert-parallel SwiGLU MLP (MoE experts) for 8 Trainium2 NeuronCores.

Problem: routed_in_egD [E*G, D] fp32, w1/w3 [E, D, F], w2 [E, F, D], E=8,
G=2048, D=2048, F=5632.  reference:
    x_egD = routed.reshape(E, G, D)
    mid   = silu(x @ w1) * (x @ w3)          # [E, G, F]
    out   = (mid @ w2).reshape(E*G, D)

Sharding: expert-parallel - core e gets expert e's x slice + weights; no
collectives.  Each core runs three 2048x2048x5632-class GEMMs (~142 GFLOP).

Per-core kernel (all matmuls bf16, PSUM fp32); ~1.90ms on HW, i.e. ~96% of
the 78.6 TF/s bf16 tensor-engine roofline (1.82ms of matmul at N=512):
  phase 0: x [G,D] DMA-cast to bf16, PE-transpose -> xT [D,G] bf16 in SBUF
           (overlapped with the HBM x stream, which is the phase bound).
  phase 1: per f-chunk (128 rows of F), per g-quarter: gateT/upT = w1/w3.T @ x
           accumulated over D, one PSUM bank per group with 4-deep rotation
           so the PE never waits on the ACT/DVE SwiGLU drain; bf16 weights
           get the fast-weight-load path so LDWEIGHTS hides under matmuls.
           SwiGLU (ACT silu -> bf16, DVE mul); midT [F,G] spilled as bf16.
  phase 2: out[g,d] = sum_f midT[f,g]*w2[f,d]: mid panels stationary (bf16),
           w2 panels DMA-cast fp32->bf16 (moving), PSUM accumulation over F;
           accumulators come from the still-live phase-1 PSUM pools to avoid
           a pool-release barrier at the phase boundary.  Output lands in
           natural [G, D] layout.
"""

import numpy as np

import concourse.mybir as mybir
import concourse.tile as tile
from concourse import bacc
from concourse.bass_utils import run_bass_kernel_spmd
from concourse.masks import make_identity

E, G, D, F = 8, 2048, 2048, 5632
P = 128
DO = D // P      # 16 d-chunks
FC = F // P      # 44 f-chunks
GO = G // P      # 16 g-chunks

F32 = mybir.dt.float32
BF16 = mybir.dt.bfloat16


def build_nc():
    nc = bacc.Bacc("TRN2", target_bir_lowering=False)
    x = nc.dram_tensor("x", [G, D], F32, kind="ExternalInput").ap()
    w1 = nc.dram_tensor("w1", [D, F], F32, kind="ExternalInput").ap()
    w2 = nc.dram_tensor("w2", [F, D], F32, kind="ExternalInput").ap()
    w3 = nc.dram_tensor("w3", [D, F], F32, kind="ExternalInput").ap()
    out = nc.dram_tensor("out", [G, D], F32, kind="ExternalOutput").ap()

    w1r = w1.rearrange("(do p) f -> p do f", p=P)
    w3r = w3.rearrange("(do p) f -> p do f", p=P)
    w2r = w2.rearrange("(fo p) d -> p fo d", p=P)

    with tile.TileContext(nc) as tc:
        dram = tc.alloc_tile_pool(name="dram", bufs=1, space="DRAM")
        # midT stored gp-blocked and f-major within each g-panel:
        # mid5[p, gp, fo, g'] = silu/up product for f = fo*128+p, g = gp*256+g'.
        # Phase-1 writes are per-partition contiguous 512B; phase-2 panel reads
        # are per-partition contiguous 22KB (128 DMA blocks instead of 5632).
        mid5 = dram.tile([P, 8, FC, 256], BF16)

        wp = tc.alloc_tile_pool(name="wp", bufs=4)
        mp = tc.alloc_tile_pool(name="mp", bufs=4)
        xtp = tc.alloc_tile_pool(name="xtp", bufs=1)
        xT = xtp.tile([P, DO, G], BF16)

        # PSUM pools for all phases, allocated up front.  Transpose tiles
        # draw from the pu tag (same 2KB bank size), so the phase-1 GATE
        # chains have no dependency on the transposes at all and start as
        # soon as the first xT quarter + w1 tile land — a separate
        # transpose pool would barrier phase 1 behind ALL transposes.
        ps1g = tc.alloc_tile_pool(name="ps1g", bufs=4, space="PSUM")
        ps1u = tc.alloc_tile_pool(name="ps1u", bufs=4, space="PSUM")

        # ---- phase 0: x [G, D] -> xT [d_in, d_out, g] (bf16)
        p0 = tc.alloc_tile_pool(name="p0", bufs=8)
        idp = tc.alloc_tile_pool(name="idp", bufs=1)
        ident = idp.tile([P, P], BF16)
        make_identity(nc, ident)

        def emit_transpose_block(go):
            for q in range(2):
                xsq = p0.tile([P, 1024], BF16, tag="xs", name="xsq")
                nc.gpsimd.dma_start(
                    xsq, x[go * P : (go + 1) * P, q * 1024 : (q + 1) * 1024]
                )
                tp = ps1u.tile([P, 8, P], BF16, tag="pu", bufs=4, name="tp")
                for j in range(8):
                    nc.tensor.transpose(tp[:, j], xsq[:, j * P : (j + 1) * P], ident)
                nc.vector.tensor_copy(
                    xT[:, q * 8 : (q + 1) * 8, go * P : (go + 1) * P], tp
                )

        # ---- phase 1: midT[f, g] = silu(w1.T x) * (w3.T x), spill bf16
        # One PSUM bank per accumulation group (g-quarter), 4-deep rotation:
        # the PE runs up to 3 groups ahead of the ACT/DVE SwiGLU drain, so a
        # late silu/mul never stalls the matmul stream.
        def emit_p1_group(fc, gq, w1t, w3t):
            gsl = slice(gq * 512, (gq + 1) * 512)
            pg = ps1g.tile([P, 512], F32, tag="pg", name="pg")
            pu = ps1u.tile([P, 512], F32, tag="pu", name="pu")
            for d in range(DO):
                st, sp_ = (d == 0), (d == DO - 1)
                nc.tensor.matmul(
                    pg, w1t[:, d], xT[:, d, gsl], start=st, stop=sp_
                )
                nc.tensor.matmul(
                    pu, w3t[:, d], xT[:, d, gsl], start=st, stop=sp_
                )
            mo = mp.tile([P, 2, 256], BF16, tag="mo")
            nc.scalar.activation(
                mo, pg.rearrange("p (j g) -> p j g", j=2),
                mybir.ActivationFunctionType.Silu,
            )
            nc.vector.tensor_mul(
                mo, mo, pu.rearrange("p (j g) -> p j g", j=2)
            )
            for j in range(2):
                nc.scalar.dma_start(mid5[:, gq * 2 + j, fc], mo[:, j])

        def load_w(fc):
            w1t = wp.tile([P, DO, P], BF16, tag="w1", name="w1t")
            nc.gpsimd.dma_start(w1t, w1r[:, :, fc * P : (fc + 1) * P])
            w3t = wp.tile([P, DO, P], BF16, tag="w3", name="w3t")
            nc.gpsimd.dma_start(w3t, w3r[:, :, fc * P : (fc + 1) * P])
            return w1t, w3t

        # fc=0 is interleaved with the transposes: the PE queue is strict
        # FIFO, so if all 256 transposes were emitted first, phase-1 matmuls
        # would sit behind transposes that are still waiting on the HBM x
        # stream.  Emitting [transposes for one g-quarter, then fc0's chains
        # for that quarter] keeps the PE busy through the x stream.
        w1t0, w3t0 = load_w(0)
        for b in range(4):
            for go in range(4 * b, 4 * b + 4):
                emit_transpose_block(go)
            emit_p1_group(0, b, w1t0, w3t0)
        idp.release()
        p0.release()

        for fc in range(1, FC):
            w1t, w3t = load_w(fc)
            for gq in range(4):
                emit_p1_group(fc, gq, w1t, w3t)
        xtp.release()
        mp.release()
        wp.release()

        # ---- phase 2: out[g, d] = midT.T @ w2 (bf16 x bf16, fp32 psum)
        # w2 quarters are split into two half-tiles so the SBUF anti-deps
        # against still-live phase-1 pools resolve per-half and the dq=0
        # prefetch can start during the phase-1 tail.
        FH = FC // 2  # 22
        # phase-2 accumulators come from the still-live phase-1 PSUM pools
        # (tags pg/pu): releasing those pools and allocating a fresh one here
        # would put a full barrier between the last phase-1 silu and the
        # first phase-2 matmul.
        w2p = tc.alloc_tile_pool(name="w2p", bufs=2, side="right")
        mqp = tc.alloc_tile_pool(name="mqp", bufs=3, side="right")
        op = tc.alloc_tile_pool(name="op", bufs=6, side="right")
        w2bounds = [0, 6, 12, 18, 22, 29, 34, 39, 44]
        for dq in range(4):
            w2h = [
                w2p.tile([P, FH, 512], BF16, tag="w2a", name="w2qa"),
                w2p.tile([P, FH, 512], BF16, tag="w2b", name="w2qb"),
            ]
            dsl = slice(dq * 512, (dq + 1) * 512)
            if dq == 0:
                # chunk by fo so the first accumulation steps can start early
                for k in range(8):
                    lo, hi = w2bounds[k], w2bounds[k + 1]
                    nc.gpsimd.dma_start(
                        w2h[lo // FH][:, lo % FH : (hi - 1) % FH + 1, :],
                        w2r[:, lo:hi, dsl],
                    )
            else:
                nc.gpsimd.dma_start(w2h[0], w2r[:, 0:FH, dsl])
                nc.gpsimd.dma_start(w2h[1], w2r[:, FH:FC, dsl])
            for gp in range(8):
                mq = mqp.tile([P, FC, 256], BF16, tag="mq")
                # dq=0 panel loads go on the otherwise-idle Sync HWDGE ring
                # (Scalar's FIFO is clogged with phase-1 mid5 writes);
                # alternate queues afterwards so consecutive loads overlap
                dma_eng = nc.sync if (dq == 0 or gp % 2 == 0) else nc.scalar
                if dq == 0 and gp == 0:
                    for k in range(8):
                        lo, hi = w2bounds[k], w2bounds[k + 1]
                        nc.sync.dma_start(
                            mq[:, lo:hi, :], mid5[:, 0, lo:hi, :]
                        )
                else:
                    dma_eng.dma_start(mq, mid5[:, gp])
                po = [
                    ps1g.tile([P, 512], F32, tag="pg", name="pog"),
                    ps1u.tile([P, 512], F32, tag="pu", name="pou"),
                ]
                for fo in range(FC):
                    st, sp_ = (fo == 0), (fo == FC - 1)
                    for gc in range(2):
                        nc.tensor.matmul(
                            po[gc],
                            mq[:, fo, gc * P : (gc + 1) * P],
                            w2h[fo // FH][:, fo % FH],
                            start=st,
                            stop=sp_,
                        )
                for gc in range(2):
                    ot = op.tile([P, 512], F32, tag="ot")
                    nc.vector.tensor_copy(ot, po[gc])
                    g0 = (gp * 2 + gc) * P
                    nc.scalar.dma_start(
                        out[g0 : g0 + P, dq * 512 : (dq + 1) * 512], ot
                    )
        op.release()
        mqp.release()
        w2p.release()
        ps1u.release()
        ps1g.release()
        dram.release()
    nc.compile()
    return nc


_NC_CACHE = None


def _get_nc():
    global _NC_CACHE
    if _NC_CACHE is None:
        _NC_CACHE = build_nc()
    return _NC_CACHE


def _in_maps(routed_in_egD, w1, w2, w3):
    x = np.ascontiguousarray(np.asarray(routed_in_egD, dtype=np.float32))
    w1 = np.ascontiguousarray(np.asarray(w1, dtype=np.float32))
    w2 = np.ascontiguousarray(np.asarray(w2, dtype=np.float32))
    w3 = np.ascontiguousarray(np.asarray(w3, dtype=np.float32))
    x_e = x.reshape(E, G, D)
    return [
        {"x": x_e[e], "w1": w1[e], "w2": w2[e], "w3": w3[e]} for e in range(E)
    ]


def kernel(routed_in_egD, w1, w2, w3):
    nc = _get_nc()
    in_maps = _in_maps(routed_in_egD, w1, w2, w3)
    # the execute occasionally dies with a transient
    # NRT_EXEC_UNIT_UNRECOVERABLE through the PJRT tunnel; retry with a
    # short backoff
    last = None
    for attempt in range(4):
        try:
            res = run_bass_kernel_spmd(nc, in_maps, core_ids=list(range(E)))
            break
        except Exception as e:
            last = e
            import time as _time

            _time.sleep(2.0 * (attempt + 1))
    else:
        raise last
    return np.concatenate([r["out"] for r in res.results], axis=0)


def run_traced(routed_in_egD, w1, w2, w3, **trace_kwargs):
    """For test.py: run with NTFF tracing; returns (full_out, BassKernelResults)."""
    nc = _get_nc()
    res = run_bass_kernel_spmd(
        nc,
        _in_maps(routed_in_egD, w1, w2, w3),
        core_ids=list(range(E)),
        trace=True,
        **trace_kwargs,
    )
    out = np.concatenate([r["out"] for r in res.results], axis=0)
    return out, res
